# revision 1
# baseline (speedup 1.0000x reference)
"""BiMamba4TS Trainium2 Bass kernel.

Full-input contract: kernel(**inputs) takes the unsharded inputs from
setup_inputs() and returns the full [8, 4, 64, 62, 1] output.

Sharding: pure data parallel over the leading batch dim B=8 -> one batch
sample per NeuronCore.  Each core:
  - computes the SRA routing decision from its correlations slice on-device
  - folds the channel_independent/channel_mixing token select INTO the PE:
    tokT = xa^T @ ((1-f) I) + xb^T @ (f I) as two accumulating normal-mode
    matmuls against runtime-scaled fp32r identities (branch-free routing;
    normal-mode matmuls also keep the PE HAM clock warm -- transpose-mode
    does not count as PE-busy and runs throttled)
  - runs fwd+bwd mamba blocks: mm1 (x@W1+b1, silu) -> 3-tap conv across
    channels (6 accumulated 128x128x512 matmuls per output tile) + silu ->
    folded output projection (W2@Wr precomputed on host, so mm2 + the final
    einsum collapse into a single [F,1] dot)
  - the bwd direction's sequence flip is pure indexing (S is a batch dim for
    everything except the final sum), folded into the dot's read pattern.

All heavy matmuls use fp32r (1 cycle/row on TRN2 when the moving free dim
is >= 256, i.e. full 78.6 TF/s with fp32 storage).
"""

import contextlib

import numpy as np

import concourse.bass as bass
import concourse.tile as tile
from concourse import bacc, mybir
from concourse.masks import make_identity

# Problem shapes (hardcoded per contract)
B = 8
N1, S, L, P, F = 4, 64, 8192, 128, 256
LP = L // 128          # 64 patches per series
FH = 128               # half of F (PE partition limit)
CB = 512               # matmul moving-dim batch (columns)
NB = (S * LP) // CB    # 8 batches of 512 cols per n
OUTL = LP - 2          # 62 valid conv outputs per patch-block
NCORES = 8

F32 = mybir.dt.float32
F32R = mybir.dt.float32r
ALU = mybir.AluOpType
ACTF = mybir.ActivationFunctionType
AXX = mybir.AxisListType.X


def build_program():
    nc = bacc.Bacc("TRN2", target_bir_lowering=False, debug=False)

    x_d = nc.dram_tensor("x", [N1, S, L], F32R, kind="ExternalInput")
    corr_d = nc.dram_tensor("corr", [S, 1024], F32, kind="ExternalInput")
    w1_d = nc.dram_tensor("w1", [2, P, F], F32, kind="ExternalInput")
    cwt_d = nc.dram_tensor("cwt", [2, 3, 2, 2, FH, FH], F32, kind="ExternalInput")
    bp_d = nc.dram_tensor("biasp", [P, 13], F32, kind="ExternalInput")
    out_d = nc.dram_tensor("out", [N1, S, OUTL], F32, kind="ExternalOutput")

    x4 = x_d.ap().rearrange("n s (lp p) -> n s lp p", p=P)  # [4, 64, 64, 128]

    with tile.TileContext(nc) as tc:
        with contextlib.ExitStack() as ctx:
            _build_body(nc, tc, ctx, x4, corr_d, w1_d, cwt_d, bp_d, out_d)
    nc.compile()
    return nc


def _build_body(nc, tc, ctx, x4, corr_d, w1_d, cwt_d, bp_d, out_d):
    const = ctx.enter_context(tc.tile_pool(name="const", bufs=1))

    # ---- resident weights -------------------------------------------------
    # (the decide correlations ride the scalar queue FIRST -- the routing
    # flag gates the conditional token DMAs, so its latency is critical)
    corr_sb = const.tile([S, 1024], F32)
    nc.scalar.dma_start(out=corr_sb, in_=corr_d.ap())
    w1_sb = const.tile([P, 2, F], F32)
    nc.scalar.dma_start(out=w1_sb, in_=w1_d.ap().rearrange("d p f -> p d f"))
    cwt_sb = const.tile([FH, 2, 3, 2, 2, FH], F32)
    nc.scalar.dma_start(
        out=cwt_sb, in_=cwt_d.ap().rearrange("d k i o fi fo -> fi d k i o fo")
    )
    bp_sb = const.tile([P, 13], F32)
    nc.scalar.dma_start(out=bp_sb, in_=bp_d.ap())
    ident = const.tile([P, P], F32)
    make_identity(nc, ident)
    # fp32r copies of all matmul weights (walrus requires fp32r matmul
    # operands to be produced -- i.e. rounded -- as fp32r)
    cwtr = const.tile([FH, 2, 3, 2, 2, FH], F32R)
    nc.vector.tensor_copy(out=cwtr, in_=cwt_sb)
    w2pr = const.tile([P, 4], F32R)
    nc.vector.tensor_copy(out=w2pr, in_=bp_sb[:, 8:12])
    zpad = const.tile([P, 2], F32)
    nc.vector.memset(zpad, 0.0)
    identr = const.tile([P, P], F32R)
    nc.vector.tensor_copy(out=identr, in_=ident)
    # routing-scaled first-layer weights (written by the decide stage):
    # w1sel[:, d, 0] = (1-f) W1_d  (channel_independent),
    # w1sel[:, d, 1] = f W1_d      (channel_mixing / permuted read)
    w1sel = const.tile([P, 2, 2, F], F32R)

    # ---- persistent hT buffers (one n in flight) --------------------------
    hpool = ctx.enter_context(tc.tile_pool(name="ht", bufs=1))
    hbuf = {}
    for d in range(2):
        for i in range(2):
            t = hpool.tile([P, NB * CB + 2], F32R, name=f"ht_{d}_{i}")
            nc.vector.tensor_copy(out=t[:, NB * CB : NB * CB + 2], in_=zpad)
            hbuf[(d, i)] = t

    xa_p = ctx.enter_context(tc.tile_pool(name="xa", bufs=8))
    tk_ps = ctx.enter_context(tc.tile_pool(name="tkps", bufs=2, space="PSUM"))
    tok_p = ctx.enter_context(tc.tile_pool(name="tok", bufs=1))
    tok2_p = ctx.enter_context(tc.tile_pool(name="tok2", bufs=1))
    mm_ps = ctx.enter_context(tc.tile_pool(name="mmps", bufs=2, space="PSUM"))
    cv_ps = ctx.enter_context(tc.tile_pool(name="cvps", bufs=3, space="PSUM"))
    sf_p = ctx.enter_context(tc.tile_pool(name="sf", bufs=14))
    dt_ps = ctx.enter_context(tc.tile_pool(name="dtps", bufs=1, space="PSUM"))
    os_p = ctx.enter_context(tc.tile_pool(name="osb", bufs=4))

    # ---- decide: SRA correlation routing ---------------------------------
    def emit_decide(dec):
        c_t = corr_sb
        csum = dec.tile([S, 1], F32)
        nc.vector.reduce_sum(out=csum, in_=c_t, axis=AXX)
        cmean = dec.tile([S, 1], F32)
        nc.vector.tensor_scalar(
            out=cmean, in0=csum, scalar1=1.0 / 1024.0, scalar2=None, op0=ALU.mult
        )
        # centered (in place)
        nc.vector.tensor_scalar(
            out=c_t, in0=c_t, scalar1=cmean, scalar2=None, op0=ALU.subtract
        )
        sq = dec.tile([S, 1024], F32)
        nc.vector.tensor_tensor(out=sq, in0=c_t, in1=c_t, op=ALU.mult)
        ssq = dec.tile([S, 1], F32)
        nc.vector.reduce_sum(out=ssq, in_=sq, axis=AXX)
        stdv = dec.tile([S, 1], F32)
        # torch.std is unbiased: std = sqrt(ssq / (L-1))
        nc.scalar.activation(out=stdv, in_=ssq, func=ACTF.Sqrt, scale=1.0 / 1023.0)
        rstd = dec.tile([S, 1], F32)
        nc.vector.reciprocal(out=rstd, in_=stdv)
        nc.vector.tensor_scalar(
            out=c_t, in0=c_t, scalar1=rstd, scalar2=None, op0=ALU.mult
        )
        # gram matrix G = norm @ norm.T  (contract 1024 via 8 transposed blocks)
        normt = dec.tile([P, 512], F32)
        for k in range(8):
            tp = tk_ps.tile([P, S], F32, tag="tab")
            nc.tensor.transpose(
                out=tp, in_=c_t[:, 128 * k : 128 * (k + 1)], identity=ident[0:S, 0:S]
            )
            nc.vector.tensor_copy(out=normt[:, S * k : S * (k + 1)], in_=tp)
        gps = tk_ps.tile([S, S], F32, tag="tab")
        for k in range(8):
            nc.tensor.matmul(
                out=gps,
                lhsT=normt[:, S * k : S * (k + 1)],
                rhs=normt[:, S * k : S * (k + 1)],
                start=(k == 0),
                stop=(k == 7),
            )
        # counts: corr > 0.6  <=>  G > 0.6*1024 ;  corr > 0  <=>  G > 0
        c1 = dec.tile([S, S], F32)
        c0 = dec.tile([S, S], F32)
        nc.vector.tensor_scalar(
            out=c1, in0=gps, scalar1=0.6 * 1024.0, scalar2=None, op0=ALU.is_gt
        )
        nc.vector.tensor_scalar(
            out=c0, in0=gps, scalar1=0.0, scalar2=None, op0=ALU.is_gt
        )
        r1 = dec.tile([S, 2], F32)
        nc.vector.reduce_sum(out=r1[:, 0:1], in_=c1, axis=AXX)
        nc.vector.reduce_sum(out=r1[:, 1:2], in_=c0, axis=AXX)
        onescol = dec.tile([S, 1], F32)
        nc.vector.memset(onescol, 1.0)
        cntps = tk_ps.tile([1, 2], F32, tag="tab")
        nc.tensor.matmul(out=cntps, lhsT=onescol, rhs=r1, start=True, stop=True)
        cnts = dec.tile([1, 2], F32)
        nc.vector.tensor_copy(out=cnts, in_=cntps)
        # ratio >= 0.4 with the diagonal (64 self-pairs) removed:
        #   (cnt_thr-64) >= 0.4*(cnt_pos-64)  <=>  cnt_thr - 0.4*cnt_pos >= 38.4
        t1 = dec.tile([1, 1], F32)
        nc.vector.tensor_scalar(
            out=t1, in0=cnts[:, 1:2], scalar1=-0.4, scalar2=None, op0=ALU.mult
        )
        t2 = dec.tile([1, 1], F32)
        nc.vector.tensor_tensor(out=t2, in0=cnts[:, 0:1], in1=t1, op=ALU.add)
        flag = dec.tile([1, 1], F32)
        nc.vector.tensor_scalar(
            out=flag, in0=t2, scalar1=38.3999, scalar2=None, op0=ALU.is_ge
        )
        # broadcast flag across partitions (K=1 matmul with a ones row)
        onesrow = dec.tile([1, P], F32)
        nc.vector.memset(onesrow, 1.0)
        fps = tk_ps.tile([P, 1], F32, tag="tab")
        nc.tensor.matmul(out=fps, lhsT=onesrow, rhs=flag, start=True, stop=True)
        fvec = dec.tile([P, 1], F32)
        nc.vector.tensor_copy(out=fvec, in_=fps)
        onemf = dec.tile([P, 1], F32)
        nc.vector.tensor_scalar(
            out=onemf, in0=fvec, scalar1=-1.0, scalar2=1.0, op0=ALU.mult, op1=ALU.add
        )
        for d in range(2):
            nc.vector.tensor_scalar(
                out=w1sel[:, d, 0, :],
                in0=w1_sb[:, d, :],
                scalar1=onemf,
                scalar2=None,
                op0=ALU.mult,
            )
            nc.vector.tensor_scalar(
                out=w1sel[:, d, 1, :],
                in0=w1_sb[:, d, :],
                scalar1=fvec,
                scalar2=None,
                op0=ALU.mult,
            )



    def emit_chunk_pair(n, cp, tokt, tokt2):
        """Load + transpose (normal-mode matmul vs fp32r identity) for chunks
        (2cp, 2cp+1): 4 s-tiles, 256 token columns.  The transposed tokens
        land twice: s-major (channel_independent mm1 read) and lp-major
        (channel_mixing mm1 read, so both selects read contiguously)."""
        tab = tk_ps.tile([P, 2, P], F32, tag="tab")  # one PSUM bank
        xa = xa_p.tile([P, 2, P], F32R)
        for j in range(2):
            c = 2 * cp + j
            eng = nc.sync if j == 0 else nc.scalar
            eng.dma_start(
                out=xa[:, j, :],
                in_=x4[n, 2 * c : 2 * c + 2].rearrange("s lp p -> (s lp) p"),
            )
            nc.tensor.matmul(
                out=tab[:, j, :], lhsT=xa[:, j, :], rhs=identr, start=True, stop=True
            )
        nc.vector.tensor_copy(
            out=tokt[:, 2 * P * cp : 2 * P * (cp + 1)].rearrange(
                "p (c h) -> p c h", h=P
            ),
            in_=tab,
        )
        for j in range(2):
            c = 2 * cp + j
            a2 = tokt2[:]
            dst = bass.AP(
                tensor=a2.tensor,
                offset=a2.offset + 2 * c,
                ap=[a2.ap[0], [1, 2], [LP, LP]],
            )
            nc.vector.tensor_copy(
                out=dst, in_=tab[:, j, :].rearrange("p (s l) -> p s l", l=LP)
            )

    def mm1_batch(n, tokt, tokt2, bi):
        for d in range(2):
            for i in range(2):
                ps = mm_ps.tile([P, CB], F32)
                nc.tensor.matmul(
                    out=ps,
                    lhsT=w1sel[:, d, 0, i * FH : (i + 1) * FH],
                    rhs=tokt[:, CB * bi : CB * (bi + 1)],
                    start=True,
                    stop=False,
                )
                nc.tensor.matmul(
                    out=ps,
                    lhsT=w1sel[:, d, 1, i * FH : (i + 1) * FH],
                    rhs=tokt2[:, CB * bi : CB * (bi + 1)],
                    start=False,
                    stop=True,
                )
                nc.scalar.activation(
                    out=hbuf[(d, i)][:, CB * bi : CB * (bi + 1)],
                    in_=ps,
                    func=ACTF.Silu,
                    bias=bp_sb[:, 2 * d + i : 2 * d + i + 1],
                    scale=1.0,
                )

    def conv_block(d, bi):
        """3-tap conv over patch positions + silu -> {fo_half: sf tile}."""
        sf = {}
        for o in range(2):
            ps = cv_ps.tile([P, CB], F32)
            first = True
            for i in range(2):
                for k in range(3):
                    nc.tensor.matmul(
                        out=ps,
                        lhsT=cwtr[:, d, k, i, o, :],
                        rhs=hbuf[(d, i)][:, CB * bi + k : CB * bi + k + CB],
                        start=first,
                        stop=(i == 1 and k == 2),
                    )
                    first = False
            t = sf_p.tile([P, CB], F32R)
            nc.scalar.activation(
                out=t,
                in_=ps,
                func=ACTF.Silu,
                bias=bp_sb[:, 4 + 2 * d + o : 5 + 2 * d + o],
                scale=1.0,
            )
            sf[o] = t
        return sf

    def flip_ap(t):
        """[128, 512] tile viewed with its 8 64-col blocks in reverse order."""
        a = t[:]
        return bass.AP(
            tensor=a.tensor,
            offset=a.offset + 7 * LP,
            ap=[a.ap[0], [-LP, 8], [1, LP]],
        )

    def dot_block(n, bi, sff, sfb):
        """Folded (W2 @ Wr) projection; bwd read s-flipped; +const; DMA out."""
        ps = dt_ps.tile([1, CB], F32)
        nc.tensor.matmul(
            out=ps, lhsT=w2pr[:, 0:1], rhs=sff[0], start=True, stop=False
        )
        nc.tensor.matmul(
            out=ps, lhsT=w2pr[:, 1:2], rhs=sff[1], start=False, stop=False
        )
        nc.tensor.matmul(
            out=ps, lhsT=w2pr[:, 2:3], rhs=flip_ap(sfb[0]), start=False, stop=False
        )
        nc.tensor.matmul(
            out=ps, lhsT=w2pr[:, 3:4], rhs=flip_ap(sfb[1]), start=False, stop=True
        )
        outs = os_p.tile([1, CB], F32)
        nc.scalar.activation(
            out=outs, in_=ps, func=ACTF.Identity, bias=bp_sb[0:1, 12:13], scale=1.0
        )
        ov = outs[:].rearrange("q (s l) -> q s l", l=LP)[:, :, 0:OUTL]
        nc.sync.dma_start(out=out_d.ap()[n, 8 * bi : 8 * bi + 8, :], in_=ov)

    def conv_phase(n, tokt_next, tokt2_next):
        """conv+dot for n, with n+1 token chunk-pairs woven in (bursts of 4
        pairs, so tok matmuls don't fragment the conv accumulation stream)."""
        k = 0

        def weave4():
            nonlocal k
            if tokt_next is not None:
                for _ in range(4):
                    emit_chunk_pair(n + 1, k, tokt_next, tokt2_next)
                    k += 1

        for half in (0, 1):
            lo = 4 * half
            sff = {}
            for b in range(lo, lo + 4):
                sff[b] = conv_block(0, b)
            weave4()
            for b in range(lo, lo + 4):
                bm = 7 - b
                sfb = conv_block(1, bm)
                dot_block(n, b, sff[b], sfb)
            weave4()

    # ---- main schedule ----------------------------------------------------
    tokts = [None] * (N1 + 1)
    tokt2s = [None] * (N1 + 1)

    def new_tok(n):
        tokts[n] = tok_p.tile([P, NB * CB], F32R, name=f"tokt{n}", tag="tokt")
        tokt2s[n] = tok2_p.tile([P, NB * CB], F32R, name=f"tokt2_{n}", tag="tokt2")

    new_tok(0)
    for cp in range(8):
        emit_chunk_pair(0, cp, tokts[0], tokt2s[0])
    # decide rides the same PSUM slots as the token transposes; its DVE chain
    # overlaps the n0 token loads, so the PE never waits on it at the start
    dec_pool = ctx.enter_context(tc.tile_pool(name="dec", bufs=1))
    emit_decide(dec_pool)
    for cp in range(8, 16):
        emit_chunk_pair(0, cp, tokts[0], tokt2s[0])
    for b in range(NB):
        mm1_batch(0, tokts[0], tokt2s[0], b)
    for n in range(N1):
        if n + 1 < N1:
            new_tok(n + 1)
        if n > 0:
            for b in range(NB):
                mm1_batch(n, tokts[n], tokt2s[n], b)
        conv_phase(n, tokts[n + 1], tokt2s[n + 1])


_PROGRAM = None


def _get_program():
    global _PROGRAM
    if _PROGRAM is None:
        _PROGRAM = build_program()
    return _PROGRAM


def _pack_weights(inputs):
    f32 = np.float32
    w1 = np.stack(
        [np.asarray(inputs["W1f"], f32), np.asarray(inputs["W1b"], f32)]
    )  # [2, P, F]
    cwt = np.empty((2, 3, 2, 2, FH, FH), f32)
    for d, key in enumerate(["Cwf", "Cwb"]):
        cw = np.asarray(inputs[key], f32)  # [F_out, F_in, 3]
        t = np.transpose(cw, (1, 0, 2))  # [fi, fo, k]
        for k in range(3):
            for i in range(2):
                for o in range(2):
                    cwt[d, k, i, o] = t[
                        i * FH : (i + 1) * FH, o * FH : (o + 1) * FH, k
                    ]
    wr = np.asarray(inputs["Wr"], f32)  # [F, 1]
    w2pf = np.asarray(inputs["W2f"], f32) @ wr  # [F, 1]
    w2pb = np.asarray(inputs["W2b"], f32) @ wr
    cconst = (
        np.asarray(inputs["b2f"], f32) @ wr
        + np.asarray(inputs["b2b"], f32) @ wr
        + np.asarray(inputs["br"], f32)
    ).item()
    bp = np.zeros((P, 13), f32)
    b1f = np.asarray(inputs["b1f"], f32)
    b1b = np.asarray(inputs["b1b"], f32)
    cbf = np.asarray(inputs["Cbf"], f32)
    cbb = np.asarray(inputs["Cbb"], f32)
    bp[:, 0] = b1f[:FH]
    bp[:, 1] = b1f[FH:]
    bp[:, 2] = b1b[:FH]
    bp[:, 3] = b1b[FH:]
    bp[:, 4] = cbf[:FH]
    bp[:, 5] = cbf[FH:]
    bp[:, 6] = cbb[:FH]
    bp[:, 7] = cbb[FH:]
    bp[:, 8] = w2pf[:FH, 0]
    bp[:, 9] = w2pf[FH:, 0]
    bp[:, 10] = w2pb[:FH, 0]
    bp[:, 11] = w2pb[FH:, 0]
    bp[:, 12] = cconst
    return w1, cwt, bp


def make_in_maps(inputs):
    x = np.ascontiguousarray(np.asarray(inputs["x"], np.float32))  # [8,4,64,8192]
    corr = np.ascontiguousarray(np.asarray(inputs["correlations"], np.float32))
    w1, cwt, bp = _pack_weights(inputs)
    return [
        {"x": x[b], "corr": corr[b], "w1": w1, "cwt": cwt, "biasp": bp}
        for b in range(NCORES)
    ]


def kernel(**inputs) -> np.ndarray:
    from concourse.bass_utils import run_bass_kernel_spmd

    nc = _get_program()
    in_maps = make_in_maps(inputs)
    res = run_bass_kernel_spmd(nc, in_maps, core_ids=list(range(NCORES)))
    out = np.stack([res.results[b]["out"] for b in range(NCORES)])
    return out[..., None].astype(np.float32)  # [8, 4, 64, 62, 1]



# revision 7
# speedup vs baseline: 1.2427x; 1.2427x over previous
"""BiMamba4TS Trainium2 Bass kernel (v2).

Full-input contract: kernel(**inputs) takes the unsharded inputs from
setup_inputs() and returns the full [8, 4, 64, 62, 1] output.

Sharding: pure data parallel over the leading batch dim B=8 -> one batch
sample per NeuronCore.

Key structure (vs the v1 baseline):
  - The SRA routing decision is computed on host (numpy) per batch sample,
    like the host-side weight folding the baseline already did.  The flag
    only selects the (s, lp) vs (lp, s) token order, so it is folded into
    the host-side transpose of x: the device program is flag-independent
    and identical on all 8 cores.
  - x is pre-transposed on host to [N1, P, S*LP] token-major layout and
    cast to bf16, so the device needs no PE transposes, no DVE casts, and
    mm1 is a single matmul per (d, i, batch) -- PE work per n is
    32 mm1 + 192 conv + 4x8 dot matmuls.
  - All matmuls are bf16 (moving 512 cols, K=M=128) accumulating fp32 in
    PSUM; silu runs on the scalar engine writing bf16 back to SBUF.
  - The final (W2 @ Wr)-folded projection is 4 concurrent M=1 matmuls on
    4 distinct PE column groups (partitions 0/32/64/96 of one PSUM bank),
    reduced by 3 vector-engine adds (the DVE is otherwise idle).
  - hbuf is double-buffered across n so mm1(n+1) weaves into conv(n)'s
    matmul stream with no PE idle gap at n boundaries (keeps HAM warm).
  - b2@Wr + br is a scalar constant added on host at the end.
"""

import contextlib

import numpy as np

import concourse.bass as bass
import concourse.tile as tile
from concourse import bacc, mybir

# Problem shapes (hardcoded per contract)
B = 8
N1, S, L, P, F = 4, 64, 8192, 128, 256
LP = L // 128          # 64 patches per series
FH = 128               # half of F (PE partition limit)
CB = 512               # matmul moving-dim batch (columns)
NB = (S * LP) // CB    # 8 batches of 512 cols per n
OUTL = LP - 2          # 62 valid conv outputs per patch-block
NCORES = 8
NTOK = S * LP          # 4096 tokens per n

F32 = mybir.dt.float32
BF16 = mybir.dt.bfloat16
ALU = mybir.AluOpType
ACTF = mybir.ActivationFunctionType


def build_program():
    nc = bacc.Bacc("TRN2", target_bir_lowering=False, debug=False)

    x_d = nc.dram_tensor("x", [N1, P, NTOK], BF16, kind="ExternalInput")
    w1_d = nc.dram_tensor("w1", [P, 2, 2, FH], BF16, kind="ExternalInput")
    cwt_d = nc.dram_tensor("cwt", [FH, 2, 3, 2, 2, FH], BF16, kind="ExternalInput")
    w2p_d = nc.dram_tensor("w2p", [P, 4], BF16, kind="ExternalInput")
    bp_d = nc.dram_tensor("biasp", [P, 8], F32, kind="ExternalInput")
    out_d = nc.dram_tensor("out", [N1, S, OUTL], F32, kind="ExternalOutput")

    with tile.TileContext(nc) as tc:
        with contextlib.ExitStack() as ctx:
            _build_body(nc, tc, ctx, x_d, w1_d, cwt_d, w2p_d, bp_d, out_d)
    nc.compile()
    return nc


def _build_body(nc, tc, ctx, x_d, w1_d, cwt_d, w2p_d, bp_d, out_d):
    const = ctx.enter_context(tc.tile_pool(name="const", bufs=1))

    # ---- resident weights (all bf16, pre-packed on host) ------------------
    w1_sb = const.tile([P, 2, 2, FH], BF16)
    nc.scalar.dma_start(out=w1_sb, in_=w1_d.ap())
    cwt_sb = const.tile([FH, 2, 3, 2, 2, FH], BF16)
    nc.scalar.dma_start(out=cwt_sb, in_=cwt_d.ap())
    w2p_sb = const.tile([P, 4], BF16)
    nc.scalar.dma_start(out=w2p_sb, in_=w2p_d.ap())
    bp_sb = const.tile([P, 8], F32)
    nc.scalar.dma_start(out=bp_sb, in_=bp_d.ap())

    # ---- persistent buffers ----------------------------------------------
    xt_p = ctx.enter_context(tc.tile_pool(name="xt", bufs=2))
    xts = [None] * (N1 + 1)

    hpool = ctx.enter_context(tc.tile_pool(name="ht", bufs=1))
    # hbuf[(set, d, i)]: [P, NTOK + 2] bf16, 2 zero pad cols for conv tail
    hbuf = {}
    for st in range(2):
        for d in range(2):
            for i in range(2):
                t = hpool.tile([P, NTOK + 2], BF16, name=f"ht_{st}_{d}_{i}")
                nc.vector.memset(t[:, NTOK : NTOK + 2], 0.0)
                hbuf[(st, d, i)] = t

    mm_ps = ctx.enter_context(tc.tile_pool(name="mmps", bufs=2, space="PSUM"))
    cv_ps = ctx.enter_context(tc.tile_pool(name="cvps", bufs=4, space="PSUM"))
    dt_ps = ctx.enter_context(tc.tile_pool(name="dtps", bufs=2, space="PSUM"))
    sff_p = ctx.enter_context(tc.tile_pool(name="sff", bufs=8))
    sfb_p = ctx.enter_context(tc.tile_pool(name="sfb", bufs=3))
    tt_p = ctx.enter_context(tc.tile_pool(name="tt", bufs=2))
    os_p = ctx.enter_context(tc.tile_pool(name="osb", bufs=2))

    outs = [None] * N1
    sff = {}

    def xt_dma(n):
        xts[n] = xt_p.tile([P, NTOK], BF16, name=f"xt{n}", tag="xt")
        nc.sync.dma_start(out=xts[n], in_=x_d.ap()[n])

    def mm1_unit(n, b, d, i):
        """h[d,i][:, 512b:512(b+1)] = silu(W1[d,i]^T @ xT + b1)  (1 MM + ACT)"""
        ps = mm_ps.tile([P, CB], F32)
        nc.tensor.matmul(
            out=ps,
            lhsT=w1_sb[:, d, i, :],
            rhs=xts[n][:, CB * b : CB * (b + 1)],
            start=True,
            stop=True,
        )
        nc.scalar.activation(
            out=hbuf[(n % 2, d, i)][:, CB * b : CB * (b + 1)],
            in_=ps,
            func=ACTF.Silu,
            bias=bp_sb[:, 2 * d + i : 2 * d + i + 1],
            scale=1.0,
        )

    def mm1_units(n):
        for b in range(NB):
            for d in range(2):
                for i in range(2):
                    yield (n, b, d, i)

    def conv_block(n, d, bi, pool):
        """3-tap conv over patch positions + silu -> sf tile [P, 2, CB] bf16."""
        sf = pool.tile([P, 2, CB], BF16)
        for o in range(2):
            ps = cv_ps.tile([P, CB], F32)
            first = True
            for i in range(2):
                for k in range(3):
                    nc.tensor.matmul(
                        out=ps,
                        lhsT=cwt_sb[:, d, k, i, o, :],
                        rhs=hbuf[(n % 2, d, i)][:, CB * bi + k : CB * bi + k + CB],
                        start=first,
                        stop=(i == 1 and k == 2),
                    )
                    first = False
            nc.scalar.activation(
                out=sf[:, o, :],
                in_=ps,
                func=ACTF.Silu,
                bias=bp_sb[:, 4 + 2 * d + o : 5 + 2 * d + o],
                scale=1.0,
            )
        return sf

    def flip_o(t, o):
        """sf tile [P, 2, CB] -> [P, 512] view of half o with its 8 64-col
        s-chunks in reverse order (the bwd direction's S flip)."""
        a = t[:]
        return bass.AP(
            tensor=a.tensor,
            offset=a.offset + o * CB + 7 * LP,
            ap=[a.ap[0], [-LP, 8], [1, LP]],
        )

    def dot_block(n, b, sfb):
        """Folded (W2 @ Wr) projection: 4 concurrent M=1 matmuls on distinct
        PE column groups, then 3 DVE adds into the out staging tile."""
        dt = dt_ps.tile([P, CB], F32)
        nc.tensor.matmul(
            out=dt[0:1, :], lhsT=w2p_sb[:, 0:1], rhs=sff[b][:, 0, :],
            start=True, stop=True, skip_group_check=True,
        )
        nc.tensor.matmul(
            out=dt[32:33, :], lhsT=w2p_sb[:, 1:2], rhs=sff[b][:, 1, :],
            start=True, stop=True, skip_group_check=True,
        )
        nc.tensor.matmul(
            out=dt[64:65, :], lhsT=w2p_sb[:, 2:3], rhs=flip_o(sfb, 0),
            start=True, stop=True, skip_group_check=True,
        )
        nc.tensor.matmul(
            out=dt[96:97, :], lhsT=w2p_sb[:, 3:4], rhs=flip_o(sfb, 1),
            start=True, stop=True, skip_group_check=True,
            tile_position=(0, 96),
        )
        # DVE may read at most one PSUM operand per instruction
        t1 = tt_p.tile([1, CB], F32)
        nc.vector.tensor_copy(out=t1, in_=dt[0:1, :])
        t2 = tt_p.tile([1, CB], F32)
        nc.vector.tensor_tensor(out=t2, in0=t1, in1=dt[32:33, :], op=ALU.add)
        t3 = tt_p.tile([1, CB], F32)
        nc.vector.tensor_tensor(out=t3, in0=t2, in1=dt[64:65, :], op=ALU.add)
        nc.vector.tensor_tensor(
            out=outs[n][:, CB * b : CB * (b + 1)],
            in0=t3,
            in1=dt[96:97, :],
            op=ALU.add,
        )

    def out_dma(n):
        ov = outs[n][:].rearrange("q (s l) -> q s l", l=LP)[:, :, 0:OUTL]
        nc.scalar.dma_start(out=out_d.ap()[n], in_=ov)

    # ---- main schedule ----------------------------------------------------
    xt_dma(0)
    xt_dma(1)

    # n=0: mm1 woven with conv-d0 by readiness (startup is ACT-bound)
    outs[0] = os_p.tile([1, NTOK], F32, name="outs0", tag="outs")
    for b in range(NB):
        for d in range(2):
            for i in range(2):
                mm1_unit(0, b, d, i)
        if b >= 1:
            sff[b - 1] = conv_block(0, 0, b - 1, sff_p)
    sff[7] = conv_block(0, 0, 7, sff_p)

    def d1_phase(n, units, per_block):
        """conv-d1 blocks bm=7..0, dots delayed one block, weave mm1 units."""
        sfb_tiles = {}
        for j in range(NB):
            bm = 7 - j
            sfb_tiles[bm] = conv_block(n, 1, bm, sfb_p)
            for _ in range(per_block):
                u = next(units, None)
                if u is not None:
                    mm1_unit(*u)
            if j >= 1:
                # dot(b = j-1) pairs sff[j-1] with sfb[7-(j-1)] = sfb[8-j]
                dot_block(n, j - 1, sfb_tiles.pop(8 - j))
        dot_block(n, 7, sfb_tiles.pop(0))
        out_dma(n)

    # all 32 mm1(1) units weave into n=0's d1 phase (4 per block)
    d1_phase(0, mm1_units(1), 4)

    for n in range(1, N1):
        outs[n] = os_p.tile([1, NTOK], F32, name=f"outs{n}", tag="outs")
        if n + 1 < N1:
            xt_dma(n + 1)
            units = mm1_units(n + 1)
        else:
            units = iter(())
        for bi in range(NB):
            sff[bi] = conv_block(n, 0, bi, sff_p)
            for _ in range(2):
                u = next(units, None)
                if u is not None:
                    mm1_unit(*u)
        d1_phase(n, units, 2)


_PROGRAM = None


def _get_program():
    global _PROGRAM
    if _PROGRAM is None:
        _PROGRAM = build_program()
    return _PROGRAM


def _decide(corr):
    """Vectorized SRA_Decider on host: bool [B]."""
    c = np.asarray(corr, np.float64)
    n = c.shape[-1]
    mean = c.mean(axis=-1, keepdims=True)
    std = c.std(axis=-1, ddof=1, keepdims=True)
    norm = (c - mean) / std
    g = np.einsum("bsl,btl->bst", norm, norm) / n
    s = g.shape[-1]
    idx = np.arange(s)
    g[:, idx, idx] = 0.0
    cnt_thr = (g > 0.6).sum(axis=(1, 2)).astype(np.float64)
    cnt_pos = (g > 0.0).sum(axis=(1, 2)).astype(np.float64)
    ratio = np.where(cnt_pos > 0, cnt_thr / np.maximum(cnt_pos, 1.0), 0.0)
    return ratio >= 0.4


def _bf16(a):
    import ml_dtypes

    return np.asarray(a, np.float32).astype(ml_dtypes.bfloat16)


def _pack_weights(inputs):
    f32 = np.float32
    w1 = np.stack(
        [np.asarray(inputs["W1f"], f32), np.asarray(inputs["W1b"], f32)], axis=1
    ).reshape(P, 2, 2, FH)  # [p, d, i, fo]
    cwt = np.empty((2, 3, 2, 2, FH, FH), f32)
    for d, key in enumerate(["Cwf", "Cwb"]):
        cw = np.asarray(inputs[key], f32)  # [F_out, F_in, 3]
        t = np.transpose(cw, (1, 0, 2))  # [fi, fo, k]
        for k in range(3):
            for i in range(2):
                for o in range(2):
                    cwt[d, k, i, o] = t[
                        i * FH : (i + 1) * FH, o * FH : (o + 1) * FH, k
                    ]
    cwt = np.ascontiguousarray(np.transpose(cwt, (4, 0, 1, 2, 3, 5)))
    wr = np.asarray(inputs["Wr"], f32)  # [F, 1]
    w2pf = np.asarray(inputs["W2f"], f32) @ wr  # [F, 1]
    w2pb = np.asarray(inputs["W2b"], f32) @ wr
    w2p = np.stack(
        [w2pf[:FH, 0], w2pf[FH:, 0], w2pb[:FH, 0], w2pb[FH:, 0]], axis=1
    )  # [P, 4]
    cconst = (
        np.asarray(inputs["b2f"], f32) @ wr
        + np.asarray(inputs["b2b"], f32) @ wr
        + np.asarray(inputs["br"], f32)
    ).item()
    bp = np.zeros((P, 8), f32)
    b1f = np.asarray(inputs["b1f"], f32)
    b1b = np.asarray(inputs["b1b"], f32)
    cbf = np.asarray(inputs["Cbf"], f32)
    cbb = np.asarray(inputs["Cbb"], f32)
    bp[:, 0] = b1f[:FH]
    bp[:, 1] = b1f[FH:]
    bp[:, 2] = b1b[:FH]
    bp[:, 3] = b1b[FH:]
    bp[:, 4] = cbf[:FH]
    bp[:, 5] = cbf[FH:]
    bp[:, 6] = cbb[:FH]
    bp[:, 7] = cbb[FH:]
    return _bf16(w1), _bf16(cwt), _bf16(w2p), bp, cconst


def make_in_maps(inputs):
    flags = _decide(np.asarray(inputs["correlations"], np.float32))
    xb = _bf16(inputs["x"]).reshape(B, N1, S, LP, P)
    w1, cwt, w2p, bp, cconst = _pack_weights(inputs)
    in_maps = []
    for b in range(NCORES):
        if flags[b]:
            # channel_mixing: token (i, j) = x[b, n, j, i*128:(i+1)*128]
            xt = np.transpose(xb[b], (0, 3, 2, 1))
        else:
            # channel_independent: token (i, j) = x[b, n, i, j*128:(j+1)*128]
            xt = np.transpose(xb[b], (0, 3, 1, 2))
        xt = np.ascontiguousarray(xt).reshape(N1, P, NTOK)
        in_maps.append({"x": xt, "w1": w1, "cwt": cwt, "w2p": w2p, "biasp": bp})
    return in_maps, cconst


def kernel(**inputs) -> np.ndarray:
    from concourse.bass_utils import run_bass_kernel_spmd

    nc = _get_program()
    in_maps, cconst = make_in_maps(inputs)
    res = run_bass_kernel_spmd(nc, in_maps, core_ids=list(range(NCORES)))
    out = np.stack([res.results[b]["out"] for b in range(NCORES)])
    return (out + cconst)[..., None].astype(np.float32)  # [8, 4, 64, 62, 1]


# revision 12
# speedup vs baseline: 1.4785x; 1.1897x over previous
"""BiMamba4TS Trainium2 Bass kernel (v3).

Full-input contract: kernel(**inputs) takes the unsharded inputs from
setup_inputs() and returns the full [8, 4, 64, 62, 1] output.

Sharding: pure data parallel over the leading batch dim B=8 -> one batch
sample per NeuronCore.

Structure:
  - The SRA routing decision is computed on host (numpy) per batch sample,
    like the host-side weight folding the baseline already did.  The flag
    only selects the (s, lp) vs (lp, s) token order, so it is folded into
    the host-side transpose of x: the device program is flag-independent
    and identical on all 8 cores.
  - x is pre-transposed on host to [N1, P, S*LP] token-major layout and
    cast to bf16; no PE transposes, no DVE casts, single-select mm1.
  - All matmuls bf16 (512 moving cols, K=M=128) accumulating fp32 PSUM.
  - Scalar-engine silu is the secondary bottleneck (8-core P0 clock), so
    mm1 and conv both batch PAIRS of 512-col tiles into 2-bank PSUM tiles
    drained by a single [128, 1024] activation (same per-partition bias).
  - The final (W2 @ Wr)-folded projection: 4 concurrent M=1 matmuls on 4
    distinct PE column groups of one PSUM bank, reduced by a 4-op DVE
    chain (one PSUM operand per op).
  - hbuf is double-buffered across n; mm1(n+1) pair-units weave into
    conv(n)'s matmul stream (no PE idle gap at n boundaries, HAM warm).
  - b2@Wr + br is a scalar constant added on host at the end.
"""

import contextlib

import numpy as np

import concourse.bass as bass
import concourse.tile as tile
from concourse import bacc, mybir

# Problem shapes (hardcoded per contract)
B = 8
N1, S, L, P, F = 4, 64, 8192, 128, 256
LP = L // 128          # 64 patches per series
FH = 128               # half of F (PE partition limit)
CB = 512               # matmul moving-dim batch (columns)
NB = (S * LP) // CB    # 8 batches of 512 cols per n
NP = NB // 2           # 4 batch-pairs per n
OUTL = LP - 2          # 62 valid conv outputs per patch-block
NCORES = 8
NTOK = S * LP          # 4096 tokens per n

F32 = mybir.dt.float32
BF16 = mybir.dt.bfloat16
ALU = mybir.AluOpType
ACTF = mybir.ActivationFunctionType


def build_program():
    nc = bacc.Bacc("TRN2", target_bir_lowering=False, debug=False)

    x_d = nc.dram_tensor("x", [N1, P, NTOK], BF16, kind="ExternalInput")
    w1_d = nc.dram_tensor("w1", [P, 2, 2, FH], BF16, kind="ExternalInput")
    cwt_d = nc.dram_tensor("cwt", [FH, 2, 3, 2, 2, FH], BF16, kind="ExternalInput")
    w2p_d = nc.dram_tensor("w2p", [P, 4], BF16, kind="ExternalInput")
    bp_d = nc.dram_tensor("biasp", [P, 8], F32, kind="ExternalInput")
    out_d = nc.dram_tensor("out", [N1, S, OUTL], F32, kind="ExternalOutput")

    with tile.TileContext(nc) as tc:
        with contextlib.ExitStack() as ctx:
            _build_body(nc, tc, ctx, x_d, w1_d, cwt_d, w2p_d, bp_d, out_d)
    nc.compile()
    return nc


def _build_body(nc, tc, ctx, x_d, w1_d, cwt_d, w2p_d, bp_d, out_d):
    const = ctx.enter_context(tc.tile_pool(name="const", bufs=1))

    # ---- resident weights (bf16, pre-packed on host) ----------------------
    # w1 rides the sync queue FIRST (the very first matmul needs it);
    # the rest load on the scalar queue in parallel.
    w1_sb = const.tile([P, 2, 2, FH], BF16)
    nc.sync.dma_start(out=w1_sb, in_=w1_d.ap())
    bp_sb = const.tile([P, 8], F32)
    nc.scalar.dma_start(out=bp_sb, in_=bp_d.ap())
    cwt_sb = const.tile([FH, 2, 3, 2, 2, FH], BF16)
    nc.scalar.dma_start(out=cwt_sb, in_=cwt_d.ap())
    w2p_sb = const.tile([P, 4], BF16)
    nc.scalar.dma_start(out=w2p_sb, in_=w2p_d.ap())

    # ---- persistent buffers ----------------------------------------------
    xt_p = ctx.enter_context(tc.tile_pool(name="xt", bufs=2))
    xts = [None] * (N1 + 1)

    hpool = ctx.enter_context(tc.tile_pool(name="ht", bufs=1))
    # hbuf[(set, d, i)]: [P, NTOK + 2] bf16, 2 zero pad cols for conv tail
    hbuf = {}
    for st in range(2):
        for d in range(2):
            for i in range(2):
                t = hpool.tile([P, NTOK + 2], BF16, name=f"ht_{st}_{d}_{i}")
                nc.vector.memset(t[:, NTOK : NTOK + 2], 0.0)
                hbuf[(st, d, i)] = t

    mm_ps = ctx.enter_context(tc.tile_pool(name="mmps", bufs=1, space="PSUM"))
    cv_ps = ctx.enter_context(tc.tile_pool(name="cvps", bufs=2, space="PSUM"))
    dt_ps = ctx.enter_context(tc.tile_pool(name="dtps", bufs=2, space="PSUM"))
    sff_p = ctx.enter_context(tc.tile_pool(name="sff", bufs=4))
    sfb_p = ctx.enter_context(tc.tile_pool(name="sfb", bufs=2))
    tt_p = ctx.enter_context(tc.tile_pool(name="tt", bufs=2))
    os_p = ctx.enter_context(tc.tile_pool(name="osb", bufs=2))

    outs = [None] * N1
    sffp = {}  # b-pair index -> sf pair tile

    def xt_dma(n, nchunks=2):
        xts[n] = xt_p.tile([P, NTOK], BF16, name=f"xt{n}", tag="xt")
        step = NTOK // nchunks
        for c in range(nchunks):
            nc.sync.dma_start(
                out=xts[n][:, c * step : (c + 1) * step],
                in_=x_d.ap()[n][:, c * step : (c + 1) * step],
            )

    def mm1_pair(n, bp, d, i):
        """h[d,i][:, 1024bp:1024(bp+1)] = silu(W1[d,i]^T @ xT + b1).

        Two same-weight matmuls into a 2-bank PSUM tile, one ACT drain."""
        ps = mm_ps.tile([P, 2, CB], F32)
        for j in range(2):
            nc.tensor.matmul(
                out=ps[:, j, :],
                lhsT=w1_sb[:, d, i, :],
                rhs=xts[n][:, CB * (2 * bp + j) : CB * (2 * bp + j + 1)],
                start=True,
                stop=True,
                skip_group_check=True,
            )
        nc.scalar.activation(
            out=hbuf[(n % 2, d, i)][:, 2 * CB * bp : 2 * CB * (bp + 1)],
            in_=ps,
            func=ACTF.Silu,
            bias=bp_sb[:, 2 * d + i : 2 * d + i + 1],
            scale=1.0,
        )

    def mm1_pairs(n):
        for bp in range(NP):
            for d in range(2):
                for i in range(2):
                    yield (n, bp, d, i)

    def conv_opass(n, d, bp, o):
        """One o-half of a conv bi-pair: 12 matmuls (6 weights x 2 bi) into
        a 2-bank PSUM tile; returns it for the ACT drain."""
        ps = cv_ps.tile([P, 2, CB], F32)
        for idx, (i, k) in enumerate([(i, k) for i in range(2) for k in range(3)]):
            for j in range(2):
                nc.tensor.matmul(
                    out=ps[:, j, :],
                    lhsT=cwt_sb[:, d, k, i, o, :],
                    rhs=hbuf[(n % 2, d, i)][
                        :, CB * (2 * bp + j) + k : CB * (2 * bp + j) + k + CB
                    ],
                    start=(idx == 0),
                    stop=(idx == 5),
                    skip_group_check=True,
                )
        return ps

    def conv_act(n, d, o, ps, sfp):
        nc.scalar.activation(
            out=sfp[:, o, :, :],
            in_=ps,
            func=ACTF.Silu,
            bias=bp_sb[:, 4 + 2 * d + o : 5 + 2 * d + o],
            scale=1.0,
        )

    def conv_pair(n, d, bp, pool, weave=()):
        """Conv bi-pair (2bp, 2bp+1): o=0 12 MMs + ACT, o=1 12 MMs + ACT.
        `weave` holds mm1 pair-units spread around the o-runs."""
        sfp = pool.tile([P, 2, 2, CB], BF16)
        weave = list(weave)
        if weave:
            mm1_pair(*weave.pop(0))
        ps0 = conv_opass(n, d, bp, 0)
        if weave:
            mm1_pair(*weave.pop(0))
        ps1 = conv_opass(n, d, bp, 1)
        conv_act(n, d, 0, ps0, sfp)
        conv_act(n, d, 1, ps1, sfp)
        for u in weave:
            mm1_pair(*u)
        return sfp

    def flip_oj(t, o, j):
        """sf pair tile [P, 2, 2, CB] -> [P, 512] view of (o, j) with its 8
        64-col s-chunks reversed (the bwd direction's S flip)."""
        a = t[:]
        return bass.AP(
            tensor=a.tensor,
            offset=a.offset + (2 * o + j) * CB + 7 * LP,
            ap=[a.ap[0], [-LP, 8], [1, LP]],
        )

    def dot_block(n, b, sfbp, jb):
        """Folded (W2 @ Wr) projection for output block b: 4 concurrent M=1
        matmuls on distinct PE column groups, then a 4-op DVE chain (one
        PSUM operand per instruction)."""
        sfft = sffp[b // 2]
        jf = b % 2
        dt = dt_ps.tile([P, CB], F32)
        nc.tensor.matmul(
            out=dt[0:1, :], lhsT=w2p_sb[:, 0:1], rhs=sfft[:, 0, jf, :],
            start=True, stop=True, skip_group_check=True,
        )
        nc.tensor.matmul(
            out=dt[32:33, :], lhsT=w2p_sb[:, 1:2], rhs=sfft[:, 1, jf, :],
            start=True, stop=True, skip_group_check=True,
        )
        nc.tensor.matmul(
            out=dt[64:65, :], lhsT=w2p_sb[:, 2:3], rhs=flip_oj(sfbp, 0, jb),
            start=True, stop=True, skip_group_check=True,
        )
        nc.tensor.matmul(
            out=dt[96:97, :], lhsT=w2p_sb[:, 3:4], rhs=flip_oj(sfbp, 1, jb),
            start=True, stop=True, skip_group_check=True,
            tile_position=(0, 96),
        )
        t1 = tt_p.tile([1, CB], F32)
        nc.vector.tensor_copy(out=t1, in_=dt[0:1, :])
        t2 = tt_p.tile([1, CB], F32)
        nc.vector.tensor_tensor(out=t2, in0=t1, in1=dt[32:33, :], op=ALU.add)
        t3 = tt_p.tile([1, CB], F32)
        nc.vector.tensor_tensor(out=t3, in0=t2, in1=dt[64:65, :], op=ALU.add)
        nc.vector.tensor_tensor(
            out=outs[n][:, CB * b : CB * (b + 1)],
            in0=t3,
            in1=dt[96:97, :],
            op=ALU.add,
        )

    def out_dma(n):
        ov = outs[n][:].rearrange("q (s l) -> q s l", l=LP)[:, :, 0:OUTL]
        nc.sync.dma_start(out=out_d.ap()[n], in_=ov)

    def take(it, k):
        got = []
        for _ in range(k):
            u = next(it, None)
            if u is not None:
                got.append(u)
        return got

    def d1_phase(n, units, per_block):
        """conv-d1 pair-blocks p=3..0; dots one pair-block late."""
        sfb_tiles = {}
        for idx, p in enumerate(range(NP - 1, -1, -1)):
            sfb_tiles[p] = conv_pair(n, 1, p, sfb_p, take(units, per_block))
            if idx >= 1:
                pq = NP - idx  # sfb pair emitted one block ago
                sp = sfb_tiles.pop(pq)
                b0 = 2 * (NP - 1 - pq)  # dot b indices for sfb pair pq
                dot_block(n, b0, sp, 1)
                dot_block(n, b0 + 1, sp, 0)
        sp = sfb_tiles.pop(0)
        dot_block(n, 2 * NP - 2, sp, 1)
        dot_block(n, 2 * NP - 1, sp, 0)
        out_dma(n)

    # ---- main schedule ----------------------------------------------------
    xt_dma(0, nchunks=4)
    xt_dma(1, nchunks=2)

    # n=0: mm1(0) woven with conv-d0 by readiness (startup is ACT-bound).
    # conv-d0 pair-block K reads d0 hbuf pairs <= K+1; conv-d1 pair-block p
    # (emitted descending) reads d1 pairs p and p+1.
    outs[0] = os_p.tile([1, NTOK], F32, name="outs0", tag="outs")
    for bp in (0, 1):
        for i in range(2):
            mm1_pair(0, bp, 0, i)
    w0 = {
        0: [(0, 2, 0, 0), (0, 2, 0, 1), (0, 3, 0, 0), (0, 3, 0, 1)],
        1: [(0, 3, 1, 0), (0, 3, 1, 1), (0, 2, 1, 0), (0, 2, 1, 1)],
        2: [(0, 1, 1, 0), (0, 1, 1, 1), (0, 0, 1, 0), (0, 0, 1, 1)],
    }
    u1 = mm1_pairs(1)
    for bp in range(NP):
        wv = w0[bp] if bp in w0 else take(u1, 4)
        sffp[bp] = conv_pair(0, 0, bp, sff_p, wv)
    # remaining 12 mm1(1) pair-units weave into n=0's d1 phase (3 per block)
    d1_phase(0, u1, 3)

    for n in range(1, N1):
        outs[n] = os_p.tile([1, NTOK], F32, name=f"outs{n}", tag="outs")
        if n + 1 < N1:
            xt_dma(n + 1)
            units = mm1_pairs(n + 1)
        else:
            units = iter(())
        for bp in range(NP):
            sffp[bp] = conv_pair(n, 0, bp, sff_p, take(units, 2))
        d1_phase(n, units, 2)


_PROGRAM = None


def _get_program():
    global _PROGRAM
    if _PROGRAM is None:
        _PROGRAM = build_program()
    return _PROGRAM


def _decide(corr):
    """Vectorized SRA_Decider on host: bool [B]."""
    c = np.asarray(corr, np.float64)
    n = c.shape[-1]
    mean = c.mean(axis=-1, keepdims=True)
    std = c.std(axis=-1, ddof=1, keepdims=True)
    norm = (c - mean) / std
    g = np.einsum("bsl,btl->bst", norm, norm) / n
    s = g.shape[-1]
    idx = np.arange(s)
    g[:, idx, idx] = 0.0
    cnt_thr = (g > 0.6).sum(axis=(1, 2)).astype(np.float64)
    cnt_pos = (g > 0.0).sum(axis=(1, 2)).astype(np.float64)
    ratio = np.where(cnt_pos > 0, cnt_thr / np.maximum(cnt_pos, 1.0), 0.0)
    return ratio >= 0.4


def _bf16(a):
    import ml_dtypes

    return np.asarray(a, np.float32).astype(ml_dtypes.bfloat16)


def _pack_weights(inputs):
    f32 = np.float32
    w1 = np.stack(
        [np.asarray(inputs["W1f"], f32), np.asarray(inputs["W1b"], f32)], axis=1
    ).reshape(P, 2, 2, FH)  # [p, d, i, fo]
    cwt = np.empty((2, 3, 2, 2, FH, FH), f32)
    for d, key in enumerate(["Cwf", "Cwb"]):
        cw = np.asarray(inputs[key], f32)  # [F_out, F_in, 3]
        t = np.transpose(cw, (1, 0, 2))  # [fi, fo, k]
        for k in range(3):
            for i in range(2):
                for o in range(2):
                    cwt[d, k, i, o] = t[
                        i * FH : (i + 1) * FH, o * FH : (o + 1) * FH, k
                    ]
    cwt = np.ascontiguousarray(np.transpose(cwt, (4, 0, 1, 2, 3, 5)))
    wr = np.asarray(inputs["Wr"], f32)  # [F, 1]
    w2pf = np.asarray(inputs["W2f"], f32) @ wr  # [F, 1]
    w2pb = np.asarray(inputs["W2b"], f32) @ wr
    w2p = np.stack(
        [w2pf[:FH, 0], w2pf[FH:, 0], w2pb[:FH, 0], w2pb[FH:, 0]], axis=1
    )  # [P, 4]
    cconst = (
        np.asarray(inputs["b2f"], f32) @ wr
        + np.asarray(inputs["b2b"], f32) @ wr
        + np.asarray(inputs["br"], f32)
    ).item()
    bp = np.zeros((P, 8), f32)
    b1f = np.asarray(inputs["b1f"], f32)
    b1b = np.asarray(inputs["b1b"], f32)
    cbf = np.asarray(inputs["Cbf"], f32)
    cbb = np.asarray(inputs["Cbb"], f32)
    bp[:, 0] = b1f[:FH]
    bp[:, 1] = b1f[FH:]
    bp[:, 2] = b1b[:FH]
    bp[:, 3] = b1b[FH:]
    bp[:, 4] = cbf[:FH]
    bp[:, 5] = cbf[FH:]
    bp[:, 6] = cbb[:FH]
    bp[:, 7] = cbb[FH:]
    return _bf16(w1), _bf16(cwt), _bf16(w2p), bp, cconst


def make_in_maps(inputs):
    flags = _decide(np.asarray(inputs["correlations"], np.float32))
    xb = _bf16(inputs["x"]).reshape(B, N1, S, LP, P)
    w1, cwt, w2p, bp, cconst = _pack_weights(inputs)
    in_maps = []
    for b in range(NCORES):
        if flags[b]:
            # channel_mixing: token (i, j) = x[b, n, j, i*128:(i+1)*128]
            xt = np.transpose(xb[b], (0, 3, 2, 1))
        else:
            # channel_independent: token (i, j) = x[b, n, i, j*128:(j+1)*128]
            xt = np.transpose(xb[b], (0, 3, 1, 2))
        xt = np.ascontiguousarray(xt).reshape(N1, P, NTOK)
        in_maps.append({"x": xt, "w1": w1, "cwt": cwt, "w2p": w2p, "biasp": bp})
    return in_maps, cconst


def kernel(**inputs) -> np.ndarray:
    from concourse.bass_utils import run_bass_kernel_spmd

    nc = _get_program()
    in_maps, cconst = make_in_maps(inputs)
    res = run_bass_kernel_spmd(nc, in_maps, core_ids=list(range(NCORES)))
    out = np.stack([res.results[b]["out"] for b in range(NCORES)])
    return (out + cconst)[..., None].astype(np.float32)  # [8, 4, 64, 62, 1]


# revision 16
# speedup vs baseline: 1.4927x; 1.0096x over previous
"""BiMamba4TS Trainium2 Bass kernel (v3).

Full-input contract: kernel(**inputs) takes the unsharded inputs from
setup_inputs() and returns the full [8, 4, 64, 62, 1] output.

Sharding: pure data parallel over the leading batch dim B=8 -> one batch
sample per NeuronCore.

Structure:
  - The SRA routing decision is computed on host (numpy) per batch sample,
    like the host-side weight folding the baseline already did.  The flag
    only selects the (s, lp) vs (lp, s) token order, so it is folded into
    the host-side transpose of x: the device program is flag-independent
    and identical on all 8 cores.
  - x is pre-transposed on host to [N1, P, S*LP] token-major layout and
    cast to bf16; no PE transposes, no DVE casts, single-select mm1.
  - All matmuls bf16 (512 moving cols, K=M=128) accumulating fp32 PSUM.
  - Scalar-engine silu is the secondary bottleneck (8-core P0 clock), so
    mm1 and conv both batch PAIRS of 512-col tiles into 2-bank PSUM tiles
    drained by a single [128, 1024] activation (same per-partition bias).
  - The final (W2 @ Wr)-folded projection: 4 concurrent M=1 matmuls on 4
    distinct PE column groups of one PSUM bank, reduced by a 4-op DVE
    chain (one PSUM operand per op).
  - hbuf is double-buffered across n; mm1(n+1) pair-units weave into
    conv(n)'s matmul stream (no PE idle gap at n boundaries, HAM warm).
  - b2@Wr + br is a scalar constant added on host at the end.
"""

import contextlib

import numpy as np

import concourse.bass as bass
import concourse.tile as tile
from concourse import bacc, mybir

# Problem shapes (hardcoded per contract)
B = 8
N1, S, L, P, F = 4, 64, 8192, 128, 256
LP = L // 128          # 64 patches per series
FH = 128               # half of F (PE partition limit)
CB = 512               # matmul moving-dim batch (columns)
NB = (S * LP) // CB    # 8 batches of 512 cols per n
NP = NB // 2           # 4 batch-pairs per n
OUTL = LP - 2          # 62 valid conv outputs per patch-block
NCORES = 8
NTOK = S * LP          # 4096 tokens per n

F32 = mybir.dt.float32
BF16 = mybir.dt.bfloat16
ALU = mybir.AluOpType
ACTF = mybir.ActivationFunctionType


def build_program():
    nc = bacc.Bacc("TRN2", target_bir_lowering=False, debug=False)

    x_d = nc.dram_tensor("x", [N1, P, NTOK], BF16, kind="ExternalInput")
    w1_d = nc.dram_tensor("w1", [P, 2, 2, FH], BF16, kind="ExternalInput")
    cwt_d = nc.dram_tensor("cwt", [FH, 2, 3, 2, 2, FH], BF16, kind="ExternalInput")
    w2p_d = nc.dram_tensor("w2p", [P, 4], BF16, kind="ExternalInput")
    bp_d = nc.dram_tensor("biasp", [P, 8], F32, kind="ExternalInput")
    out_d = nc.dram_tensor("out", [N1, S, OUTL], F32, kind="ExternalOutput")

    with tile.TileContext(nc) as tc:
        with contextlib.ExitStack() as ctx:
            _build_body(nc, tc, ctx, x_d, w1_d, cwt_d, w2p_d, bp_d, out_d)
    nc.compile()
    return nc


def _build_body(nc, tc, ctx, x_d, w1_d, cwt_d, w2p_d, bp_d, out_d):
    const = ctx.enter_context(tc.tile_pool(name="const", bufs=1))

    # ---- resident weights (bf16, pre-packed on host) ----------------------
    # w1 rides the sync queue FIRST (the very first matmul needs it);
    # the rest load on the scalar queue in parallel.
    w1_sb = const.tile([P, 2, 2, FH], BF16)
    nc.sync.dma_start(out=w1_sb, in_=w1_d.ap())
    bp_sb = const.tile([P, 8], F32)
    nc.scalar.dma_start(out=bp_sb, in_=bp_d.ap())
    cwt_sb = const.tile([FH, 2, 3, 2, 2, FH], BF16)
    nc.scalar.dma_start(out=cwt_sb, in_=cwt_d.ap())
    w2p_sb = const.tile([P, 4], BF16)
    nc.scalar.dma_start(out=w2p_sb, in_=w2p_d.ap())

    # ---- persistent buffers ----------------------------------------------
    xt_p = ctx.enter_context(tc.tile_pool(name="xt", bufs=2))
    xts = [None] * (N1 + 1)

    hpool = ctx.enter_context(tc.tile_pool(name="ht", bufs=1))
    # hbuf[(set, d, i)]: [P, NTOK + 2] bf16, 2 zero pad cols for conv tail
    hbuf = {}
    for st in range(2):
        for d in range(2):
            for i in range(2):
                t = hpool.tile([P, NTOK + 2], BF16, name=f"ht_{st}_{d}_{i}")
                nc.vector.memset(t[:, NTOK : NTOK + 2], 0.0)
                hbuf[(st, d, i)] = t

    mm_ps = ctx.enter_context(tc.tile_pool(name="mmps", bufs=1, space="PSUM"))
    cv_ps = ctx.enter_context(tc.tile_pool(name="cvps", bufs=2, space="PSUM"))
    dt_ps = ctx.enter_context(tc.tile_pool(name="dtps", bufs=2, space="PSUM"))
    sff_p = ctx.enter_context(tc.tile_pool(name="sff", bufs=4))
    sfb_p = ctx.enter_context(tc.tile_pool(name="sfb", bufs=2))
    tt_p = ctx.enter_context(tc.tile_pool(name="tt", bufs=2))
    os_p = ctx.enter_context(tc.tile_pool(name="osb", bufs=2))

    outs = [None] * N1
    sffp = {}  # b-pair index -> sf pair tile

    def xt_dma(n, nchunks=2):
        xts[n] = xt_p.tile([P, NTOK], BF16, name=f"xt{n}", tag="xt")
        step = NTOK // nchunks
        for c in range(nchunks):
            nc.sync.dma_start(
                out=xts[n][:, c * step : (c + 1) * step],
                in_=x_d.ap()[n][:, c * step : (c + 1) * step],
            )

    def mm1_pair(n, bp, d, i):
        """h[d,i][:, 1024bp:1024(bp+1)] = silu(W1[d,i]^T @ xT + b1).

        Two same-weight matmuls into a 2-bank PSUM tile, one ACT drain."""
        ps = mm_ps.tile([P, 2, CB], F32)
        for j in range(2):
            nc.tensor.matmul(
                out=ps[:, j, :],
                lhsT=w1_sb[:, d, i, :],
                rhs=xts[n][:, CB * (2 * bp + j) : CB * (2 * bp + j + 1)],
                start=True,
                stop=True,
                skip_group_check=True,
            )
        nc.scalar.activation(
            out=hbuf[(n % 2, d, i)][:, 2 * CB * bp : 2 * CB * (bp + 1)],
            in_=ps,
            func=ACTF.Silu,
            bias=bp_sb[:, 2 * d + i : 2 * d + i + 1],
            scale=1.0,
        )

    def mm1_pairs(n):
        for bp in range(NP):
            for d in range(2):
                for i in range(2):
                    yield (n, bp, d, i)

    def conv_opass(n, d, bp, o):
        """One o-half of a conv bi-pair: 12 matmuls (6 weights x 2 bi) into
        a 2-bank PSUM tile; returns it for the ACT drain."""
        ps = cv_ps.tile([P, 2, CB], F32)
        for idx, (i, k) in enumerate([(i, k) for i in range(2) for k in range(3)]):
            for j in range(2):
                nc.tensor.matmul(
                    out=ps[:, j, :],
                    lhsT=cwt_sb[:, d, k, i, o, :],
                    rhs=hbuf[(n % 2, d, i)][
                        :, CB * (2 * bp + j) + k : CB * (2 * bp + j) + k + CB
                    ],
                    start=(idx == 0),
                    stop=(idx == 5),
                    skip_group_check=True,
                )
        return ps

    def conv_act(n, d, o, ps, sfp):
        nc.scalar.activation(
            out=sfp[:, o, :, :],
            in_=ps,
            func=ACTF.Silu,
            bias=bp_sb[:, 4 + 2 * d + o : 5 + 2 * d + o],
            scale=1.0,
        )

    def conv_pair(n, d, bp, pool, weave=()):
        """Conv bi-pair (2bp, 2bp+1): o=0 12 MMs + ACT, o=1 12 MMs + ACT.
        `weave` holds mm1 pair-units spread around the o-runs."""
        sfp = pool.tile([P, 2, 2, CB], BF16)
        weave = list(weave)
        if weave:
            mm1_pair(*weave.pop(0))
        ps0 = conv_opass(n, d, bp, 0)
        if weave:
            mm1_pair(*weave.pop(0))
        ps1 = conv_opass(n, d, bp, 1)
        conv_act(n, d, 0, ps0, sfp)
        conv_act(n, d, 1, ps1, sfp)
        for u in weave:
            mm1_pair(*u)
        return sfp

    def flip_oj(t, o, j):
        """sf pair tile [P, 2, 2, CB] -> [P, 512] view of (o, j) with its 8
        64-col s-chunks reversed (the bwd direction's S flip)."""
        a = t[:]
        return bass.AP(
            tensor=a.tensor,
            offset=a.offset + (2 * o + j) * CB + 7 * LP,
            ap=[a.ap[0], [-LP, 8], [1, LP]],
        )

    def dot_block(n, b, sfbp, jb, act_copy=False):
        """Folded (W2 @ Wr) projection for output block b: 4 concurrent M=1
        matmuls on distinct PE column groups, then a 4-op reduce chain (one
        PSUM operand per instruction; first op on ScalarE when the DVE is
        the local bottleneck)."""
        sfft = sffp[b // 2]
        jf = b % 2
        dt = dt_ps.tile([P, CB], F32)
        nc.tensor.matmul(
            out=dt[0:1, :], lhsT=w2p_sb[:, 0:1], rhs=sfft[:, 0, jf, :],
            start=True, stop=True, skip_group_check=True,
        )
        nc.tensor.matmul(
            out=dt[32:33, :], lhsT=w2p_sb[:, 1:2], rhs=sfft[:, 1, jf, :],
            start=True, stop=True, skip_group_check=True,
        )
        nc.tensor.matmul(
            out=dt[64:65, :], lhsT=w2p_sb[:, 2:3], rhs=flip_oj(sfbp, 0, jb),
            start=True, stop=True, skip_group_check=True,
        )
        nc.tensor.matmul(
            out=dt[96:97, :], lhsT=w2p_sb[:, 3:4], rhs=flip_oj(sfbp, 1, jb),
            start=True, stop=True, skip_group_check=True,
            tile_position=(0, 96),
        )
        t1 = tt_p.tile([1, CB], F32)
        if act_copy:
            nc.scalar.activation(out=t1, in_=dt[0:1, :], func=ACTF.Copy, bias=0.0)
        else:
            nc.vector.tensor_copy(out=t1, in_=dt[0:1, :])
        t2 = tt_p.tile([1, CB], F32)
        nc.vector.tensor_tensor(out=t2, in0=t1, in1=dt[32:33, :], op=ALU.add)
        t3 = tt_p.tile([1, CB], F32)
        nc.vector.tensor_tensor(out=t3, in0=t2, in1=dt[64:65, :], op=ALU.add)
        nc.vector.tensor_tensor(
            out=outs[n][:, CB * b : CB * (b + 1)],
            in0=t3,
            in1=dt[96:97, :],
            op=ALU.add,
        )

    def out_dma(n):
        ov = outs[n][:].rearrange("q (s l) -> q s l", l=LP)[:, :, 0:OUTL]
        nc.sync.dma_start(out=out_d.ap()[n], in_=ov)

    def take(it, k):
        got = []
        for _ in range(k):
            u = next(it, None)
            if u is not None:
                got.append(u)
        return got

    def d1_phase(n, units, per_block):
        """conv-d1 pair-blocks p=3..0; dots one pair-block late."""
        sfb_tiles = {}
        for idx, p in enumerate(range(NP - 1, -1, -1)):
            sfb_tiles[p] = conv_pair(n, 1, p, sfb_p, take(units, per_block))
            if idx >= 1:
                pq = NP - idx  # sfb pair emitted one block ago
                sp = sfb_tiles.pop(pq)
                b0 = 2 * (NP - 1 - pq)  # dot b indices for sfb pair pq
                dot_block(n, b0, sp, 1)
                dot_block(n, b0 + 1, sp, 0)
        sp = sfb_tiles.pop(0)
        dot_block(n, 2 * NP - 2, sp, 1)
        dot_block(n, 2 * NP - 1, sp, 0)
        out_dma(n)

    # ---- main schedule ----------------------------------------------------
    xt_dma(0, nchunks=1)
    xt_dma(1, nchunks=1)

    # n=0: mm1(0) woven with conv-d0 by readiness (startup is ACT-bound).
    # conv-d0 pair-block K reads d0 hbuf pairs <= K+1; conv-d1 pair-block p
    # (emitted descending) reads d1 pairs p and p+1.
    outs[0] = os_p.tile([1, NTOK], F32, name="outs0", tag="outs")
    for bp in (0, 1):
        for i in range(2):
            mm1_pair(0, bp, 0, i)
    w0 = {
        0: [(0, 2, 0, 0), (0, 2, 0, 1), (0, 3, 0, 0), (0, 3, 0, 1)],
        1: [(0, 3, 1, 0), (0, 3, 1, 1), (0, 2, 1, 0), (0, 2, 1, 1)],
        2: [(0, 1, 1, 0), (0, 1, 1, 1), (0, 0, 1, 0), (0, 0, 1, 1)],
    }
    u1 = mm1_pairs(1)
    for bp in range(NP):
        wv = w0[bp] if bp in w0 else take(u1, 4)
        sffp[bp] = conv_pair(0, 0, bp, sff_p, wv)
    # remaining 12 mm1(1) pair-units weave into n=0's d1 phase (3 per block)
    d1_phase(0, u1, 3)

    # n = 1..3: interleaved segments [conv-d0 q | dots(q-1) | conv-d1 3-q];
    # dot pair q needs d0 pair q and d1 pair 3-q, both from segment q, and
    # is emitted mid-segment q+1 so its rhs ACTs have a full block of slack.
    for n in range(1, N1):
        outs[n] = os_p.tile([1, NTOK], F32, name=f"outs{n}", tag="outs")
        if n + 1 < N1:
            xt_dma(n + 1)
            units = mm1_pairs(n + 1)
        else:
            units = iter(())
        sfb_tiles = {}
        for q in range(NP):
            sffp[q] = conv_pair(n, 0, q, sff_p, take(units, 2))
            if q >= 1:
                sp = sfb_tiles.pop(NP - q)
                dot_block(n, 2 * (q - 1), sp, 1)
                dot_block(n, 2 * q - 1, sp, 0)
            sfb_tiles[NP - 1 - q] = conv_pair(n, 1, NP - 1 - q, sfb_p, take(units, 2))
        sp = sfb_tiles.pop(0)
        last = n == N1 - 1
        dot_block(n, 2 * NP - 2, sp, 1, act_copy=last)
        dot_block(n, 2 * NP - 1, sp, 0, act_copy=last)
        out_dma(n)


_PROGRAM = None


def _get_program():
    global _PROGRAM
    if _PROGRAM is None:
        _PROGRAM = build_program()
    return _PROGRAM


def _decide(corr):
    """Vectorized SRA_Decider on host: bool [B]."""
    c = np.asarray(corr, np.float64)
    n = c.shape[-1]
    mean = c.mean(axis=-1, keepdims=True)
    std = c.std(axis=-1, ddof=1, keepdims=True)
    norm = (c - mean) / std
    g = np.einsum("bsl,btl->bst", norm, norm) / n
    s = g.shape[-1]
    idx = np.arange(s)
    g[:, idx, idx] = 0.0
    cnt_thr = (g > 0.6).sum(axis=(1, 2)).astype(np.float64)
    cnt_pos = (g > 0.0).sum(axis=(1, 2)).astype(np.float64)
    ratio = np.where(cnt_pos > 0, cnt_thr / np.maximum(cnt_pos, 1.0), 0.0)
    return ratio >= 0.4


def _bf16(a):
    import ml_dtypes

    return np.asarray(a, np.float32).astype(ml_dtypes.bfloat16)


def _pack_weights(inputs):
    f32 = np.float32
    w1 = np.stack(
        [np.asarray(inputs["W1f"], f32), np.asarray(inputs["W1b"], f32)], axis=1
    ).reshape(P, 2, 2, FH)  # [p, d, i, fo]
    cwt = np.empty((2, 3, 2, 2, FH, FH), f32)
    for d, key in enumerate(["Cwf", "Cwb"]):
        cw = np.asarray(inputs[key], f32)  # [F_out, F_in, 3]
        t = np.transpose(cw, (1, 0, 2))  # [fi, fo, k]
        for k in range(3):
            for i in range(2):
                for o in range(2):
                    cwt[d, k, i, o] = t[
                        i * FH : (i + 1) * FH, o * FH : (o + 1) * FH, k
                    ]
    cwt = np.ascontiguousarray(np.transpose(cwt, (4, 0, 1, 2, 3, 5)))
    wr = np.asarray(inputs["Wr"], f32)  # [F, 1]
    w2pf = np.asarray(inputs["W2f"], f32) @ wr  # [F, 1]
    w2pb = np.asarray(inputs["W2b"], f32) @ wr
    w2p = np.stack(
        [w2pf[:FH, 0], w2pf[FH:, 0], w2pb[:FH, 0], w2pb[FH:, 0]], axis=1
    )  # [P, 4]
    cconst = (
        np.asarray(inputs["b2f"], f32) @ wr
        + np.asarray(inputs["b2b"], f32) @ wr
        + np.asarray(inputs["br"], f32)
    ).item()
    bp = np.zeros((P, 8), f32)
    b1f = np.asarray(inputs["b1f"], f32)
    b1b = np.asarray(inputs["b1b"], f32)
    cbf = np.asarray(inputs["Cbf"], f32)
    cbb = np.asarray(inputs["Cbb"], f32)
    bp[:, 0] = b1f[:FH]
    bp[:, 1] = b1f[FH:]
    bp[:, 2] = b1b[:FH]
    bp[:, 3] = b1b[FH:]
    bp[:, 4] = cbf[:FH]
    bp[:, 5] = cbf[FH:]
    bp[:, 6] = cbb[:FH]
    bp[:, 7] = cbb[FH:]
    return _bf16(w1), _bf16(cwt), _bf16(w2p), bp, cconst


def make_in_maps(inputs):
    flags = _decide(np.asarray(inputs["correlations"], np.float32))
    xb = _bf16(inputs["x"]).reshape(B, N1, S, LP, P)
    w1, cwt, w2p, bp, cconst = _pack_weights(inputs)
    in_maps = []
    for b in range(NCORES):
        if flags[b]:
            # channel_mixing: token (i, j) = x[b, n, j, i*128:(i+1)*128]
            xt = np.transpose(xb[b], (0, 3, 2, 1))
        else:
            # channel_independent: token (i, j) = x[b, n, i, j*128:(j+1)*128]
            xt = np.transpose(xb[b], (0, 3, 1, 2))
        xt = np.ascontiguousarray(xt).reshape(N1, P, NTOK)
        in_maps.append({"x": xt, "w1": w1, "cwt": cwt, "w2p": w2p, "biasp": bp})
    return in_maps, cconst


def kernel(**inputs) -> np.ndarray:
    from concourse.bass_utils import run_bass_kernel_spmd

    nc = _get_program()
    in_maps, cconst = make_in_maps(inputs)
    res = run_bass_kernel_spmd(nc, in_maps, core_ids=list(range(NCORES)))
    out = np.stack([res.results[b]["out"] for b in range(NCORES)])
    return (out + cconst)[..., None].astype(np.float32)  # [8, 4, 64, 62, 1]


# revision 17
# speedup vs baseline: 1.5098x; 1.0115x over previous
"""BiMamba4TS Trainium2 Bass kernel (v3).

Full-input contract: kernel(**inputs) takes the unsharded inputs from
setup_inputs() and returns the full [8, 4, 64, 62, 1] output.

Sharding: pure data parallel over the leading batch dim B=8 -> one batch
sample per NeuronCore.

Structure:
  - The SRA routing decision is computed on host (numpy) per batch sample,
    like the host-side weight folding the baseline already did.  The flag
    only selects the (s, lp) vs (lp, s) token order, so it is folded into
    the host-side transpose of x: the device program is flag-independent
    and identical on all 8 cores.
  - x is pre-transposed on host to [N1, P, S*LP] token-major layout and
    cast to bf16; no PE transposes, no DVE casts, single-select mm1.
  - All matmuls bf16 (512 moving cols, K=M=128) accumulating fp32 PSUM.
  - Scalar-engine silu is the secondary bottleneck (8-core P0 clock), so
    mm1 and conv both batch PAIRS of 512-col tiles into 2-bank PSUM tiles
    drained by a single [128, 1024] activation (same per-partition bias).
  - The final (W2 @ Wr)-folded projection: 4 concurrent M=1 matmuls on 4
    distinct PE column groups of one PSUM bank, reduced by a 4-op DVE
    chain (one PSUM operand per op).
  - hbuf is double-buffered across n; mm1(n+1) pair-units weave into
    conv(n)'s matmul stream (no PE idle gap at n boundaries, HAM warm).
  - b2@Wr + br is a scalar constant added on host at the end.
"""

import contextlib

import numpy as np

import concourse.bass as bass
import concourse.tile as tile
from concourse import bacc, mybir

# Problem shapes (hardcoded per contract)
B = 8
N1, S, L, P, F = 4, 64, 8192, 128, 256
LP = L // 128          # 64 patches per series
FH = 128               # half of F (PE partition limit)
CB = 512               # matmul moving-dim batch (columns)
NB = (S * LP) // CB    # 8 batches of 512 cols per n
NP = NB // 2           # 4 batch-pairs per n
OUTL = LP - 2          # 62 valid conv outputs per patch-block
NCORES = 8
NTOK = S * LP          # 4096 tokens per n

F32 = mybir.dt.float32
BF16 = mybir.dt.bfloat16
ALU = mybir.AluOpType
ACTF = mybir.ActivationFunctionType


def build_program():
    nc = bacc.Bacc("TRN2", target_bir_lowering=False, debug=False)

    x_d = nc.dram_tensor("x", [N1, P, NTOK], BF16, kind="ExternalInput")
    w1_d = nc.dram_tensor("w1", [P, 2, 2, FH], BF16, kind="ExternalInput")
    cwt_d = nc.dram_tensor("cwt", [FH, 2, 3, 2, 2, FH], BF16, kind="ExternalInput")
    w2p_d = nc.dram_tensor("w2p", [P, 4], BF16, kind="ExternalInput")
    bp_d = nc.dram_tensor("biasp", [P, 8], F32, kind="ExternalInput")
    out_d = nc.dram_tensor("out", [N1, S, OUTL], F32, kind="ExternalOutput")

    with tile.TileContext(nc) as tc:
        with contextlib.ExitStack() as ctx:
            _build_body(nc, tc, ctx, x_d, w1_d, cwt_d, w2p_d, bp_d, out_d)
    nc.compile()
    return nc


def _build_body(nc, tc, ctx, x_d, w1_d, cwt_d, w2p_d, bp_d, out_d):
    const = ctx.enter_context(tc.tile_pool(name="const", bufs=1))

    # ---- resident weights (bf16, pre-packed on host) ----------------------
    # w1 rides the sync queue FIRST (the very first matmul needs it);
    # the rest load on the scalar queue in parallel.
    w1_sb = const.tile([P, 2, 2, FH], BF16)
    nc.sync.dma_start(out=w1_sb, in_=w1_d.ap())
    bp_sb = const.tile([P, 8], F32)
    nc.scalar.dma_start(out=bp_sb, in_=bp_d.ap())
    cwt_sb = const.tile([FH, 2, 3, 2, 2, FH], BF16)
    nc.scalar.dma_start(out=cwt_sb, in_=cwt_d.ap())
    w2p_sb = const.tile([P, 4], BF16)
    nc.scalar.dma_start(out=w2p_sb, in_=w2p_d.ap())

    # ---- persistent buffers ----------------------------------------------
    xt_p = ctx.enter_context(tc.tile_pool(name="xt", bufs=2))
    xts = [None] * (N1 + 1)

    hpool = ctx.enter_context(tc.tile_pool(name="ht", bufs=1))
    # hbuf[(set, d, i)]: [P, NTOK + 2] bf16, 2 zero pad cols for conv tail
    hbuf = {}
    for st in range(2):
        for d in range(2):
            for i in range(2):
                t = hpool.tile([P, NTOK + 2], BF16, name=f"ht_{st}_{d}_{i}")
                nc.vector.memset(t[:, NTOK : NTOK + 2], 0.0)
                hbuf[(st, d, i)] = t

    mm_ps = ctx.enter_context(tc.tile_pool(name="mmps", bufs=1, space="PSUM"))
    cv_ps = ctx.enter_context(tc.tile_pool(name="cvps", bufs=2, space="PSUM"))
    dt_ps = ctx.enter_context(tc.tile_pool(name="dtps", bufs=2, space="PSUM"))
    sff_p = ctx.enter_context(tc.tile_pool(name="sff", bufs=4))
    sfb_p = ctx.enter_context(tc.tile_pool(name="sfb", bufs=2))
    tt_p = ctx.enter_context(tc.tile_pool(name="tt", bufs=2))
    os_p = ctx.enter_context(tc.tile_pool(name="osb", bufs=2))

    outs = [None] * N1
    sffp = {}  # b-pair index -> sf pair tile

    def xt_dma(n, nchunks=2):
        xts[n] = xt_p.tile([P, NTOK], BF16, name=f"xt{n}", tag="xt")
        step = NTOK // nchunks
        for c in range(nchunks):
            nc.sync.dma_start(
                out=xts[n][:, c * step : (c + 1) * step],
                in_=x_d.ap()[n][:, c * step : (c + 1) * step],
            )

    def mm1_pair(n, bp, d, i):
        """h[d,i][:, 1024bp:1024(bp+1)] = silu(W1[d,i]^T @ xT + b1).

        Two same-weight matmuls into a 2-bank PSUM tile, one ACT drain."""
        ps = mm_ps.tile([P, 2, CB], F32)
        for j in range(2):
            nc.tensor.matmul(
                out=ps[:, j, :],
                lhsT=w1_sb[:, d, i, :],
                rhs=xts[n][:, CB * (2 * bp + j) : CB * (2 * bp + j + 1)],
                start=True,
                stop=True,
                skip_group_check=True,
            )
        nc.scalar.activation(
            out=hbuf[(n % 2, d, i)][:, 2 * CB * bp : 2 * CB * (bp + 1)],
            in_=ps,
            func=ACTF.Silu,
            bias=bp_sb[:, 2 * d + i : 2 * d + i + 1],
            scale=1.0,
        )

    def mm1_pairs(n):
        for bp in range(NP):
            for d in range(2):
                for i in range(2):
                    yield (n, bp, d, i)

    def conv_opass(n, d, bp, o):
        """One o-half of a conv bi-pair: 12 matmuls (6 weights x 2 bi) into
        a 2-bank PSUM tile; returns it for the ACT drain."""
        ps = cv_ps.tile([P, 2, CB], F32)
        for idx, (i, k) in enumerate([(i, k) for i in range(2) for k in range(3)]):
            for j in range(2):
                nc.tensor.matmul(
                    out=ps[:, j, :],
                    lhsT=cwt_sb[:, d, k, i, o, :],
                    rhs=hbuf[(n % 2, d, i)][
                        :, CB * (2 * bp + j) + k : CB * (2 * bp + j) + k + CB
                    ],
                    start=(idx == 0),
                    stop=(idx == 5),
                    skip_group_check=True,
                )
        return ps

    def conv_act(n, d, o, ps, sfp):
        nc.scalar.activation(
            out=sfp[:, o, :, :],
            in_=ps,
            func=ACTF.Silu,
            bias=bp_sb[:, 4 + 2 * d + o : 5 + 2 * d + o],
            scale=1.0,
        )

    def conv_pair(n, d, bp, pool, weave=()):
        """Conv bi-pair (2bp, 2bp+1): o=0 12 MMs + ACT, o=1 12 MMs + ACT.
        `weave` holds mm1 pair-units spread around the o-runs."""
        sfp = pool.tile([P, 2, 2, CB], BF16)
        weave = list(weave)
        if weave:
            mm1_pair(*weave.pop(0))
        ps0 = conv_opass(n, d, bp, 0)
        if weave:
            mm1_pair(*weave.pop(0))
        ps1 = conv_opass(n, d, bp, 1)
        conv_act(n, d, 0, ps0, sfp)
        conv_act(n, d, 1, ps1, sfp)
        for u in weave:
            mm1_pair(*u)
        return sfp

    def flip_oj(t, o, j):
        """sf pair tile [P, 2, 2, CB] -> [P, 512] view of (o, j) with its 8
        64-col s-chunks reversed (the bwd direction's S flip)."""
        a = t[:]
        return bass.AP(
            tensor=a.tensor,
            offset=a.offset + (2 * o + j) * CB + 7 * LP,
            ap=[a.ap[0], [-LP, 8], [1, LP]],
        )

    def dot_block(n, b, sfbp, jb, act_copy=False):
        """Folded (W2 @ Wr) projection for output block b: 4 concurrent M=1
        matmuls on distinct PE column groups, then a 4-op reduce chain (one
        PSUM operand per instruction; first op on ScalarE when the DVE is
        the local bottleneck)."""
        sfft = sffp[b // 2]
        jf = b % 2
        dt = dt_ps.tile([P, CB], F32)
        nc.tensor.matmul(
            out=dt[0:1, :], lhsT=w2p_sb[:, 0:1], rhs=sfft[:, 0, jf, :],
            start=True, stop=True, skip_group_check=True,
        )
        nc.tensor.matmul(
            out=dt[32:33, :], lhsT=w2p_sb[:, 1:2], rhs=sfft[:, 1, jf, :],
            start=True, stop=True, skip_group_check=True,
        )
        nc.tensor.matmul(
            out=dt[64:65, :], lhsT=w2p_sb[:, 2:3], rhs=flip_oj(sfbp, 0, jb),
            start=True, stop=True, skip_group_check=True,
        )
        nc.tensor.matmul(
            out=dt[96:97, :], lhsT=w2p_sb[:, 3:4], rhs=flip_oj(sfbp, 1, jb),
            start=True, stop=True, skip_group_check=True,
            tile_position=(0, 96),
        )
        t1 = tt_p.tile([1, CB], F32)
        if act_copy:
            # final-pair fast path: split the reduce across ScalarE + DVE +
            # GpSimd so the kernel tail is not DVE-serial
            nc.scalar.activation(out=t1, in_=dt[0:1, :], func=ACTF.Copy, bias=0.0)
            t2 = tt_p.tile([1, CB], F32)
            nc.vector.tensor_tensor(out=t2, in0=t1, in1=dt[32:33, :], op=ALU.add)
            t3 = tt_p.tile([1, CB], F32)
            nc.scalar.activation(out=t3, in_=dt[64:65, :], func=ACTF.Copy, bias=0.0)
            t4 = tt_p.tile([1, CB], F32)
            nc.vector.tensor_tensor(out=t4, in0=t3, in1=dt[96:97, :], op=ALU.add)
            nc.gpsimd.tensor_tensor(
                out=outs[n][:, CB * b : CB * (b + 1)], in0=t2, in1=t4, op=ALU.add
            )
            return
        nc.vector.tensor_copy(out=t1, in_=dt[0:1, :])
        t2 = tt_p.tile([1, CB], F32)
        nc.vector.tensor_tensor(out=t2, in0=t1, in1=dt[32:33, :], op=ALU.add)
        t3 = tt_p.tile([1, CB], F32)
        nc.vector.tensor_tensor(out=t3, in0=t2, in1=dt[64:65, :], op=ALU.add)
        nc.vector.tensor_tensor(
            out=outs[n][:, CB * b : CB * (b + 1)],
            in0=t3,
            in1=dt[96:97, :],
            op=ALU.add,
        )

    def out_dma(n):
        ov = outs[n][:].rearrange("q (s l) -> q s l", l=LP)[:, :, 0:OUTL]
        nc.sync.dma_start(out=out_d.ap()[n], in_=ov)

    def take(it, k):
        got = []
        for _ in range(k):
            u = next(it, None)
            if u is not None:
                got.append(u)
        return got

    def d1_phase(n, units, per_block):
        """conv-d1 pair-blocks p=3..0; dots one pair-block late."""
        sfb_tiles = {}
        for idx, p in enumerate(range(NP - 1, -1, -1)):
            sfb_tiles[p] = conv_pair(n, 1, p, sfb_p, take(units, per_block))
            if idx >= 1:
                pq = NP - idx  # sfb pair emitted one block ago
                sp = sfb_tiles.pop(pq)
                b0 = 2 * (NP - 1 - pq)  # dot b indices for sfb pair pq
                dot_block(n, b0, sp, 1)
                dot_block(n, b0 + 1, sp, 0)
        sp = sfb_tiles.pop(0)
        dot_block(n, 2 * NP - 2, sp, 1)
        dot_block(n, 2 * NP - 1, sp, 0)
        out_dma(n)

    # ---- main schedule ----------------------------------------------------
    xt_dma(0, nchunks=4)
    xt_dma(1, nchunks=1)

    # n=0: mm1(0) woven with conv-d0 by readiness (startup is ACT-bound).
    # conv-d0 pair-block K reads d0 hbuf pairs <= K+1; conv-d1 pair-block p
    # (emitted descending) reads d1 pairs p and p+1.
    outs[0] = os_p.tile([1, NTOK], F32, name="outs0", tag="outs")
    for i in range(2):
        for bp in (0, 1):
            mm1_pair(0, bp, 0, i)
    w0 = {
        0: [(0, 2, 0, 0), (0, 2, 0, 1), (0, 3, 0, 0), (0, 3, 0, 1)],
        1: [(0, 3, 1, 0), (0, 3, 1, 1), (0, 2, 1, 0), (0, 2, 1, 1)],
        2: [(0, 1, 1, 0), (0, 1, 1, 1), (0, 0, 1, 0), (0, 0, 1, 1)],
    }
    u1 = mm1_pairs(1)
    for bp in range(NP):
        wv = w0[bp] if bp in w0 else take(u1, 4)
        sffp[bp] = conv_pair(0, 0, bp, sff_p, wv)
    # remaining 12 mm1(1) pair-units weave into n=0's d1 phase (3 per block)
    d1_phase(0, u1, 3)

    # n = 1..3: interleaved segments [conv-d0 q | dots(q-1) | conv-d1 3-q];
    # dot pair q needs d0 pair q and d1 pair 3-q, both from segment q, and
    # is emitted mid-segment q+1 so its rhs ACTs have a full block of slack.
    for n in range(1, N1):
        outs[n] = os_p.tile([1, NTOK], F32, name=f"outs{n}", tag="outs")
        if n + 1 < N1:
            xt_dma(n + 1)
            units = mm1_pairs(n + 1)
        else:
            units = iter(())
        sfb_tiles = {}
        for q in range(NP):
            sffp[q] = conv_pair(n, 0, q, sff_p, take(units, 2))
            if q >= 1:
                sp = sfb_tiles.pop(NP - q)
                dot_block(n, 2 * (q - 1), sp, 1)
                dot_block(n, 2 * q - 1, sp, 0)
            sfb_tiles[NP - 1 - q] = conv_pair(n, 1, NP - 1 - q, sfb_p, take(units, 2))
        sp = sfb_tiles.pop(0)
        last = n == N1 - 1
        dot_block(n, 2 * NP - 2, sp, 1, act_copy=last)
        dot_block(n, 2 * NP - 1, sp, 0, act_copy=last)
        out_dma(n)


_PROGRAM = None


def _get_program():
    global _PROGRAM
    if _PROGRAM is None:
        _PROGRAM = build_program()
    return _PROGRAM


def _decide(corr):
    """Vectorized SRA_Decider on host: bool [B]."""
    c = np.asarray(corr, np.float64)
    n = c.shape[-1]
    mean = c.mean(axis=-1, keepdims=True)
    std = c.std(axis=-1, ddof=1, keepdims=True)
    norm = (c - mean) / std
    g = np.einsum("bsl,btl->bst", norm, norm) / n
    s = g.shape[-1]
    idx = np.arange(s)
    g[:, idx, idx] = 0.0
    cnt_thr = (g > 0.6).sum(axis=(1, 2)).astype(np.float64)
    cnt_pos = (g > 0.0).sum(axis=(1, 2)).astype(np.float64)
    ratio = np.where(cnt_pos > 0, cnt_thr / np.maximum(cnt_pos, 1.0), 0.0)
    return ratio >= 0.4


def _bf16(a):
    import ml_dtypes

    return np.asarray(a, np.float32).astype(ml_dtypes.bfloat16)


def _pack_weights(inputs):
    f32 = np.float32
    w1 = np.stack(
        [np.asarray(inputs["W1f"], f32), np.asarray(inputs["W1b"], f32)], axis=1
    ).reshape(P, 2, 2, FH)  # [p, d, i, fo]
    cwt = np.empty((2, 3, 2, 2, FH, FH), f32)
    for d, key in enumerate(["Cwf", "Cwb"]):
        cw = np.asarray(inputs[key], f32)  # [F_out, F_in, 3]
        t = np.transpose(cw, (1, 0, 2))  # [fi, fo, k]
        for k in range(3):
            for i in range(2):
                for o in range(2):
                    cwt[d, k, i, o] = t[
                        i * FH : (i + 1) * FH, o * FH : (o + 1) * FH, k
                    ]
    cwt = np.ascontiguousarray(np.transpose(cwt, (4, 0, 1, 2, 3, 5)))
    wr = np.asarray(inputs["Wr"], f32)  # [F, 1]
    w2pf = np.asarray(inputs["W2f"], f32) @ wr  # [F, 1]
    w2pb = np.asarray(inputs["W2b"], f32) @ wr
    w2p = np.stack(
        [w2pf[:FH, 0], w2pf[FH:, 0], w2pb[:FH, 0], w2pb[FH:, 0]], axis=1
    )  # [P, 4]
    cconst = (
        np.asarray(inputs["b2f"], f32) @ wr
        + np.asarray(inputs["b2b"], f32) @ wr
        + np.asarray(inputs["br"], f32)
    ).item()
    bp = np.zeros((P, 8), f32)
    b1f = np.asarray(inputs["b1f"], f32)
    b1b = np.asarray(inputs["b1b"], f32)
    cbf = np.asarray(inputs["Cbf"], f32)
    cbb = np.asarray(inputs["Cbb"], f32)
    bp[:, 0] = b1f[:FH]
    bp[:, 1] = b1f[FH:]
    bp[:, 2] = b1b[:FH]
    bp[:, 3] = b1b[FH:]
    bp[:, 4] = cbf[:FH]
    bp[:, 5] = cbf[FH:]
    bp[:, 6] = cbb[:FH]
    bp[:, 7] = cbb[FH:]
    return _bf16(w1), _bf16(cwt), _bf16(w2p), bp, cconst


def make_in_maps(inputs):
    flags = _decide(np.asarray(inputs["correlations"], np.float32))
    xb = _bf16(inputs["x"]).reshape(B, N1, S, LP, P)
    w1, cwt, w2p, bp, cconst = _pack_weights(inputs)
    in_maps = []
    for b in range(NCORES):
        if flags[b]:
            # channel_mixing: token (i, j) = x[b, n, j, i*128:(i+1)*128]
            xt = np.transpose(xb[b], (0, 3, 2, 1))
        else:
            # channel_independent: token (i, j) = x[b, n, i, j*128:(j+1)*128]
            xt = np.transpose(xb[b], (0, 3, 1, 2))
        xt = np.ascontiguousarray(xt).reshape(N1, P, NTOK)
        in_maps.append({"x": xt, "w1": w1, "cwt": cwt, "w2p": w2p, "biasp": bp})
    return in_maps, cconst


def kernel(**inputs) -> np.ndarray:
    from concourse.bass_utils import run_bass_kernel_spmd

    nc = _get_program()
    in_maps, cconst = make_in_maps(inputs)
    res = run_bass_kernel_spmd(nc, in_maps, core_ids=list(range(NCORES)))
    out = np.stack([res.results[b]["out"] for b in range(NCORES)])
    return (out + cconst)[..., None].astype(np.float32)  # [8, 4, 64, 62, 1]
